# revision 1
# baseline (speedup 1.0000x reference)
"""GridRNN (2D recurrence) Trainium2 Bass kernel.

Sharding: data-parallel over batch (B=8 -> 8 cores, zero collectives).
Per core: for each depth, wavefront over the I+J-1 anti-diagonals of the
48x48 grid. The recurrent state for a diagonal lives pre-transposed
([2H, R] feature-major slabs, zero-padded columns at both ends so the
i=0 / j=0 boundary states are free). Per diagonal step:
  - 8 projection matmuls (input projection folded directly into the same
    PSUM accumulation as the recurrence -- no materialized p tensors)
  - 16 recurrence matmuls (state stationary, W_hh streaming, N=512)
  - tanh on ACT -> h [R, 1024] in SBUF
  - 8 PE-transposes of h packed into one PSUM bank -> next diagonal's slabs
Depth handoff (x_in/y_in of depth d+1 = hx/hy of depth d) goes through a
DRAM scratch in diagonal-major layout, consumed by the next depth's
projection matmuls (already transposed, so it is a straight DMA).
"""
import sys
sys.path.insert(0, "/opt/trn_rl_repo")
import numpy as np
import concourse.bass as bass
import concourse.tile as tile
from concourse import bacc, mybir
from concourse import bass_utils

FP32 = mybir.dt.float32
B, I, J, H, D = 8, 48, 48, 512, 3
H2 = 2 * H
NK2, NK1 = H2 // 128, H // 128  # 8, 4
ND = I + J - 1                  # 95
SLOT = 48                       # scratch cols per diagonal
TANH = mybir.ActivationFunctionType.Tanh

_cache = {}


def _build(has_bias: bool):
    nc = bacc.Bacc("TRN2", target_bir_lowering=False, debug=False, num_devices=B)
    srcT_d = nc.dram_tensor("srcT", [H, I], FP32, kind="ExternalInput")
    trgTr_d = nc.dram_tensor("trgTr", [H, J], FP32, kind="ExternalInput")
    wxh_d = nc.dram_tensor("wxh", [D, NK2, 128, H], FP32, kind="ExternalInput")
    wyh_d = nc.dram_tensor("wyh", [D, NK2, 128, H], FP32, kind="ExternalInput")
    wxi_d = nc.dram_tensor("wxi", [D, NK1, 128, H], FP32, kind="ExternalInput")
    wyi_d = nc.dram_tensor("wyi", [D, NK1, 128, H], FP32, kind="ExternalInput")
    idn_d = nc.dram_tensor("idn", [128, 128], FP32, kind="ExternalInput")
    if has_bias:
        bsx_d = nc.dram_tensor("bsx", [D, H], FP32, kind="ExternalInput")
        bsy_d = nc.dram_tensor("bsy", [D, H], FP32, kind="ExternalInput")
        ones_d = nc.dram_tensor("ones", [1, I], FP32, kind="ExternalInput")
    out_d = nc.dram_tensor("out", [D, I, J, 2, H], FP32, kind="ExternalOutput")

    outv = out_d.ap().rearrange("dp i j two h -> dp (i j) two h")

    with tile.TileContext(nc) as tc:
        with (
            tc.tile_pool(name="const", bufs=1) as constp,
            tc.tile_pool(name="wp", bufs=2) as wp,
            tc.tile_pool(name="slab", bufs=3) as slabp,
            tc.tile_pool(name="proj", bufs=4) as projp,
            tc.tile_pool(name="hsb", bufs=3) as hsbp,
            tc.tile_pool(name="pre", bufs=2, space="PSUM") as prep,
            tc.tile_pool(name="tpp", bufs=2, space="PSUM") as tpp,
            tc.tile_pool(name="scr", bufs=1, space="DRAM") as scrp,
        ):
            idn = constp.tile([128, 128], FP32, tag="idn")
            nc.sync.dma_start(idn[:], idn_d.ap())
            srcT = constp.tile([128, NK1, I], FP32, tag="srcT")
            nc.sync.dma_start(srcT[:], srcT_d.ap().rearrange("(c p) i -> p c i", p=128))
            trgTr = constp.tile([128, NK1, J], FP32, tag="trgTr")
            nc.sync.dma_start(trgTr[:], trgTr_d.ap().rearrange("(c p) j -> p c j", p=128))
            if has_bias:
                ones = constp.tile([1, I], FP32, tag="ones")
                nc.sync.dma_start(ones[:], ones_d.ap())

            # DRAM scratch for depth handoff (tracked by Tile): depth d writes
            # scr[d], depth d+1 reads scr[d].
            scr = []  # [(hx, hy)] per depth boundary
            for b in range(D - 1):
                sx = scrp.tile([NK1, 128, ND * SLOT], FP32, tag=f"sx{b}")
                sy = scrp.tile([NK1, 128, ND * SLOT], FP32, tag=f"sy{b}")
                scr.append((sx[:].rearrange("c p n -> p c n"),
                            sy[:].rearrange("c p n -> p c n")))

            def diag_geom(t):
                i_lo = max(0, t - (J - 1))
                i_hi = min(t, I - 1)
                return i_lo, i_hi - i_lo + 1

            for d in range(D):
                wxh = wp.tile([128, NK2, H], FP32, tag="wxh")
                nc.sync.dma_start(wxh[:], wxh_d.ap().rearrange("d c p n -> d p c n")[d])
                wyh = wp.tile([128, NK2, H], FP32, tag="wyh")
                nc.sync.dma_start(wyh[:], wyh_d.ap().rearrange("d c p n -> d p c n")[d])
                wxi = wp.tile([128, NK1, H], FP32, tag="wxi")
                nc.sync.dma_start(wxi[:], wxi_d.ap().rearrange("d c p n -> d p c n")[d])
                wyi = wp.tile([128, NK1, H], FP32, tag="wyi")
                nc.sync.dma_start(wyi[:], wyi_d.ap().rearrange("d c p n -> d p c n")[d])
                if has_bias:
                    bsx = wp.tile([1, H], FP32, tag="bsx")
                    nc.sync.dma_start(bsx[:], bsx_d.ap()[d:d + 1, :])
                    bsy = wp.tile([1, H], FP32, tag="bsy")
                    nc.sync.dma_start(bsy[:], bsy_d.ap()[d:d + 1, :])

                def emit_proj(t):
                    """Projection matmuls for diag t into a fresh psum tile.
                    These do not depend on the current depth's recurrence, so
                    emitting them right after diag t-1's recurrence lets the
                    PE stay busy while ACT runs tanh."""
                    i_lo, R = diag_geom(t)
                    pre = prep.tile([I, H2], FP32, tag="pre")
                    if d == 0:
                        j0 = (J - 1) - t + i_lo
                        for k in range(NK1):
                            nc.tensor.matmul(pre[0:R, 0:H], srcT[:, k, i_lo:i_lo + R],
                                             wxi[:, k, :], start=(k == 0), stop=False)
                            nc.tensor.matmul(pre[0:R, H:H2], trgTr[:, k, j0:j0 + R],
                                             wyi[:, k, :], start=(k == 0), stop=False)
                    else:
                        px = projp.tile([128, NK1, SLOT], FP32, tag="px")
                        nc.sync.dma_start(px[:, :, 0:R],
                                          scr[d - 1][0][:, :, t * SLOT:t * SLOT + R])
                        py = projp.tile([128, NK1, SLOT], FP32, tag="py")
                        nc.sync.dma_start(py[:, :, 0:R],
                                          scr[d - 1][1][:, :, t * SLOT:t * SLOT + R])
                        for k in range(NK1):
                            nc.tensor.matmul(pre[0:R, 0:H], px[:, k, 0:R],
                                             wxi[:, k, :], start=(k == 0), stop=False)
                            nc.tensor.matmul(pre[0:R, H:H2], py[:, k, 0:R],
                                             wyi[:, k, :], start=(k == 0), stop=False)
                    if has_bias:
                        nc.tensor.matmul(pre[0:R, 0:H], ones[0:1, 0:R], bsx[:],
                                         start=False, stop=False)
                        nc.tensor.matmul(pre[0:R, H:H2], ones[0:1, 0:R], bsy[:],
                                         start=False, stop=False)
                    return pre

                # zero slabs standing in for diag -1
                hxp = slabp.tile([128, NK1, SLOT + 2], FP32, tag="hx")
                hyp = slabp.tile([128, NK1, SLOT + 2], FP32, tag="hy")
                nc.vector.memset(hxp[:], 0.0)
                nc.vector.memset(hyp[:], 0.0)

                pre_next = emit_proj(0)
                for t in range(ND):
                    i_lo, R = diag_geom(t)
                    off = i_lo - max(0, t - J)  # shift vs previous diag's slab
                    pre = pre_next

                    # recurrence: state = [hx_above ; hy_left], pre += state @ Whh
                    for k in range(NK2):
                        if k < NK1:
                            st = hxp[:, k, off:off + R]
                        else:
                            st = hyp[:, k - NK1, off + 1:off + 1 + R]
                        last = (k == NK2 - 1)
                        nc.tensor.matmul(pre[0:R, 0:H], st, wxh[:, k, :],
                                         start=False, stop=last)
                        nc.tensor.matmul(pre[0:R, H:H2], st, wyh[:, k, :],
                                         start=False, stop=last)

                    if t + 1 < ND:
                        pre_next = emit_proj(t + 1)

                    h = hsbp.tile([I, H2], FP32, tag="h")
                    nc.scalar.activation(h[0:R, 0:H], pre[0:R, 0:H], TANH)
                    nc.scalar.activation(h[0:R, H:H2], pre[0:R, H:H2], TANH)

                    # write outputs: rows are cells (i, t-i), linear id stride J-1
                    row0 = i_lo * (J - 1) + t
                    sl = slice(row0, row0 + (J - 1) * (R - 1) + 1, J - 1)
                    nc.sync.dma_start(outv[d, sl, 0, :], h[0:R, 0:H])
                    nc.sync.dma_start(outv[d, sl, 1, :], h[0:R, H:H2])

                    # transpose h into next slabs (8 x [R,128] -> one PSUM bank)
                    tp = tpp.tile([128, 512], FP32, tag="tp")
                    for k in range(NK2):
                        nc.tensor.transpose(tp[:, 64 * k:64 * k + R],
                                            h[0:R, 128 * k:128 * (k + 1)],
                                            idn[0:R, 0:R])
                    hxn = slabp.tile([128, NK1, SLOT + 2], FP32, tag="hx")
                    hyn = slabp.tile([128, NK1, SLOT + 2], FP32, tag="hy")
                    tpv = tp[:].rearrange("p (c w) -> p c w", w=64)
                    nc.vector.tensor_copy(hxn[:, :, 1:R + 1], tpv[:, 0:NK1, 0:R])
                    nc.vector.tensor_copy(hyn[:, :, 1:R + 1], tpv[:, NK1:NK2, 0:R])
                    nc.vector.memset(hxn[:, :, 0:1], 0.0)
                    nc.vector.memset(hxn[:, :, R + 1:R + 2], 0.0)
                    nc.vector.memset(hyn[:, :, 0:1], 0.0)
                    nc.vector.memset(hyn[:, :, R + 1:R + 2], 0.0)

                    if d < D - 1:
                        nc.sync.dma_start(scr[d][0][:, :, t * SLOT:t * SLOT + R],
                                          hxn[:, :, 1:R + 1])
                        nc.sync.dma_start(scr[d][1][:, :, t * SLOT:t * SLOT + R],
                                          hyn[:, :, 1:R + 1])
                    hxp, hyp = hxn, hyn
    nc.compile()
    return nc


def kernel(**inputs):
    src = np.ascontiguousarray(np.asarray(inputs["src_seq_batch"], dtype=np.float32))
    trg = np.ascontiguousarray(np.asarray(inputs["trg_seq_batch"], dtype=np.float32))
    Wx_ih = np.asarray(inputs["Wx_ih"], dtype=np.float32)
    Wx_hh = np.asarray(inputs["Wx_hh"], dtype=np.float32)
    Wy_ih = np.asarray(inputs["Wy_ih"], dtype=np.float32)
    Wy_hh = np.asarray(inputs["Wy_hh"], dtype=np.float32)
    bsx = (np.asarray(inputs["bx_ih"], np.float32)
           + np.asarray(inputs["bx_hh"], np.float32))
    bsy = (np.asarray(inputs["by_ih"], np.float32)
           + np.asarray(inputs["by_hh"], np.float32))
    depth = int(np.asarray(inputs["depth"]))
    assert depth == D and src.shape == (B, I, H) and trg.shape == (B, J, H)
    has_bias = bool(np.any(bsx) or np.any(bsy))

    if has_bias not in _cache:
        _cache[has_bias] = _build(has_bias)
    nc = _cache[has_bias]

    wxh = np.ascontiguousarray(Wx_hh.reshape(D, NK2, 128, H))
    wyh = np.ascontiguousarray(Wy_hh.reshape(D, NK2, 128, H))
    wxi = np.ascontiguousarray(Wx_ih.reshape(D, NK1, 128, H))
    wyi = np.ascontiguousarray(Wy_ih.reshape(D, NK1, 128, H))
    idn = np.eye(128, dtype=np.float32)

    in_maps = []
    for c in range(B):
        m = {
            "srcT": np.ascontiguousarray(src[c].T),
            "trgTr": np.ascontiguousarray(trg[c].T[:, ::-1]),
            "wxh": wxh, "wyh": wyh, "wxi": wxi, "wyi": wyi, "idn": idn,
        }
        if has_bias:
            m["bsx"] = np.ascontiguousarray(bsx)
            m["bsy"] = np.ascontiguousarray(bsy)
            m["ones"] = np.ones((1, I), dtype=np.float32)
        in_maps.append(m)

    res = bass_utils.run_bass_kernel_spmd(nc, in_maps, list(range(B)))
    return np.stack([res.results[c]["out"] for c in range(B)], axis=0)
